# revision 26
# baseline (speedup 1.0000x reference)
"""Trainium2 Bass kernel for nn_Conv2dMem (bit-slice fake-quantized 3x3 conv).

Math (per image): unfold 3x3/pad1 -> per-row granule (32 along K, C-major)
symmetric int7 fake-quant of activations; per 32x32 block fake-quant of
weights; GEMM; bias.

Strategy (8 cores, batch-parallel, 1 image/core), v2 "all-f16 pipeline":
  - Weights fake-quantized exactly on host (numpy), fed as fp16 GEMM tiles.
  - Image stays in (C x 58*58) padded layout; conv = 18 shifted GEMM
    accumulations (2 channel-tiles x 9 kernel positions) into PSUM.
  - Granule absmax m[g, l] built on-device from image-domain max algebra
    (A/H2/H3/V2/V3 window maxes, all f16 DVE at 2x rate) + contiguous-slab
    partition-strided gather DMAs into granule space (72, 56*58) + max tree.
  - Scales rj = 63*recip(m), sj = m/63 (f16), expanded granule->channel ONCE
    per j-parity (j=0 and j=1 states) via 0/1 E-matmuls + scalar-engine evac.
    As j advances, only 4 channels per 128 change granule row; those rows are
    patched with small strided DMAs (granule row stride 9 -> channel stride 32).
  - Quantize per (j, ct, half-L): t = x*rj (f16 tensor_tensor, 2x),
    q = (t+1536)-1536 (f16 tensor_scalar, 4x; f16 magic RNE round),
    xdq = q*sj (f16 tensor_tensor, 2x). xdq stored per idx; GEMM runs nh=0
    inline and nh=1 as a sweep at half end so PSUM fits in 4 banks and the
    next half's scale expansion can overlap.
"""
import numpy as np
from contextlib import ExitStack

C_IN = 256
N_OUT = 256
H = W = 56
HP = WP = 58
NPW = HP * WP            # 3364
L = H * W                # 3136
KS = 3
GRAN = 32
NG = (C_IN * KS * KS) // GRAN   # 72 granules
MAXQ = 63.0
F16C = 1536.0            # f16 magic rounding constant (RNE to integer)
NCT = 2                  # channel partition tiles (256/128)
NH = 2                   # output-channel halves
M_CLAMP = 6e-5           # fp16-safe clamp for zero-granule guard

MCOLS = H * HP           # 3248: m/scale arrays, 58-pitch rows x 56 rows
SLAB = MCOLS - 2         # 3246: slab gather length (stays in-bounds)
HROWS = 28               # image rows per half
HALF_MC = HROWS * HP     # 1624 scale cols per half
HALF_L = HROWS * W       # 1568 packed output cols per half
SUB = 392                # PSUM sub-chunk (7 rows of 56)
NSUB = 4
EXP_P = 406              # expansion piece cols (1624/4)

ARR_NAMES = ("A", "H2", "H3", "V2", "V3")


# --------------------------------------------------------------------------
# host-side index tables
# --------------------------------------------------------------------------
def granule_terms():
    """For each g' in [0,9): list of (c'', arr, da, db) whose pointwise max
    over terms equals the granule absmax. Padded to 6 terms (repeats)."""
    out = []
    for gp in range(9):
        c_lo, c_hi = (32 * gp) // 9, (32 * gp + 31) // 9
        j0 = 32 * gp - 9 * c_lo
        j1 = 32 * gp + 32 - 9 * c_hi
        terms = []
        if j0 == 0:
            terms.append((c_lo, "V3", 0, 0))
        else:
            dh0, dw0 = divmod(j0, 3)
            terms.append((c_lo, {0: "H3", 1: "H2", 2: "A"}[dw0], dh0, dw0))
            if dh0 == 0:
                terms.append((c_lo, "V2", 1, 0))
            elif dh0 == 1:
                terms.append((c_lo, "H3", 2, 0))
        for c in range(c_lo + 1, c_hi):
            terms.append((c, "V3", 0, 0))
        if j1 == 9:
            terms.append((c_hi, "V3", 0, 0))
        else:
            q, rr = divmod(j1, 3)
            if q == 1:
                terms.append((c_hi, "H3", 0, 0))
            elif q == 2:
                terms.append((c_hi, "V2", 0, 0))
            if rr == 1:
                terms.append((c_hi, "A", q, 0))
            elif rr == 2:
                terms.append((c_hi, "H2", q, 0))
        assert 1 <= len(terms) <= 6, (gp, terms)
        while len(terms) < 6:
            terms.append(terms[0])
        out.append(terms)
    return out


TERMS = granule_terms()


def flip_table():
    """For jp in 1..8: (c*, r*): channels c* + 32t flip to granule row
    r* + 9t (+36ct via the ct free dim) when j reaches jp."""
    out = {}
    for jp in range(1, 9):
        cstar = (-25 * jp) % 32          # 9^-1 = 25 (mod 32)
        rstar = (9 * cstar + jp) // 32
        assert (9 * cstar + jp) % 32 == 0 and 0 <= rstar < 9
        out[jp] = (cstar, rstar)
    return out


FLIPS = flip_table()


def build_E2():
    """E[jst, ct] : (36, 128) f16; expands granule rows to channel rows for
    the j=jst state."""
    E = np.zeros((2, NCT, 36, 128), np.float16)
    for jst in range(2):
        for ct in range(NCT):
            for p in range(128):
                g = (9 * (128 * ct + p) + jst) // 32
                E[jst, ct, g - 36 * ct, p] = 1.0
    return E


def quantize_weight_host(weight):
    """Exact numpy replica of reference _fake_quant_weight on w2d=(K,N)."""
    w2d = weight.reshape(N_OUT, -1).T.astype(np.float32)      # (2304, 256)
    K, N = w2d.shape
    wg = w2d.reshape(K // 32, 32, N // 32, 32)
    max_abs = np.max(np.abs(wg), axis=(1, 3), keepdims=True)
    scale = (max_abs / np.float32(MAXQ)).astype(np.float32)
    scale = np.where(scale == 0, np.float32(1.0), scale)
    q = np.clip(np.round(wg / scale), -MAXQ, MAXQ)
    deq = (q * scale).astype(np.float32).reshape(K, N)
    return deq


def pack_weights(wdq):
    """(2304, 256) -> W[idx=ct*9+j, nh, p, n] fp16 stationary tiles."""
    Wt = np.zeros((NCT * 9, NH, 128, 128), np.float16)
    for ct in range(NCT):
        for j in range(9):
            rows = (9 * (128 * ct + np.arange(128)) + j)      # (128,)
            for nh in range(NH):
                Wt[ct * 9 + j, nh] = wdq[rows][:, 128 * nh:128 * nh + 128]
    return Wt


def pad_image(x):
    """(256,56,56) fp32 -> (2,128,3364) fp16 padded."""
    xp = np.pad(x, ((0, 0), (1, 1), (1, 1))).astype(np.float16)
    return xp.reshape(NCT, 128, NPW)


def make_in_maps(input, weight, bias):
    wdq = quantize_weight_host(np.asarray(weight, np.float32))
    Wt = np.ascontiguousarray(np.transpose(pack_weights(wdq), (2, 0, 1, 3))
                              ).reshape(128, NCT * 9 * NH * 128)
    E = np.ascontiguousarray(np.transpose(build_E2(), (2, 0, 1, 3))
                             ).reshape(36, 2 * NCT * 128)
    b = np.ascontiguousarray(np.asarray(bias, np.float32).reshape(NH, 128).T
                             ).astype(np.float32)
    x32 = np.asarray(input, np.float32)
    return [{"xpad": pad_image(x32[bi]), "wt": Wt, "et": E, "bias": b}
            for bi in range(x32.shape[0])]


# --------------------------------------------------------------------------
# numpy model of the device pipeline (for validation in test.py)
# --------------------------------------------------------------------------
def model_core(x, Wt, bias):
    """Bit-accurate-ish numpy model of the bass kernel for one image.
    x: (256,56,56) fp32. Returns (256,56,56) fp32."""
    f16 = np.float16
    xp16 = pad_image(x).reshape(C_IN, HP, WP)                # f16
    A = np.abs(xp16)
    H2 = np.maximum(A[:, :, :-1], A[:, :, 1:])               # (C,58,57)
    H3 = np.maximum(H2[:, :, :-1], A[:, :, 2:])              # (C,58,56)
    V2 = np.maximum(H3[:, :-1], H3[:, 1:])                   # (C,57,56)
    V3 = np.maximum(V2[:, :-1], H3[:, 2:])                   # (C,56,56)
    arrs = {"A": A, "H2": H2, "H3": H3, "V2": V2, "V3": V3}

    m = np.zeros((NG, H, W), f16)
    for g in range(NG):
        B, gp = divmod(g, 9)
        acc = None
        for (cpp, arr, da, db) in TERMS[gp]:
            c = 32 * B + cpp
            v = arrs[arr][c, da:da + H, db:db + W]
            acc = v if acc is None else np.maximum(acc, v)
        m[g] = acc

    m32 = np.maximum(m.astype(np.float32), np.float32(M_CLAMP))
    rj = (np.float32(MAXQ) * (1.0 / m32)).astype(f16)        # (72,56,56)
    sj = np.maximum(m.astype(np.float32) * np.float32(1.0 / MAXQ),
                    np.float32(M_CLAMP / MAXQ)).astype(f16)

    out = np.zeros((N_OUT, L), np.float32)
    for ct in range(NCT):
        for j in range(9):
            dh, dw = divmod(j, 3)
            cs = np.arange(128 * ct, 128 * ct + 128)
            g = (9 * cs + j) // 32                            # (128,)
            xv = xp16[cs, dh:dh + H, dw:dw + W].reshape(128, L)
            # f16 product, f16 magic round, f16 dequant -- matches device
            t = (xv * rj[g].reshape(128, L)).astype(f16)
            u = (t.astype(f16) + f16(F16C)).astype(f16)
            q = (u.astype(np.float32) - np.float32(F16C)).astype(f16)
            xdq = (q.astype(np.float32) * sj[g].reshape(128, L).astype(np.float32)).astype(f16)
            for nh in range(NH):
                Wtile = Wt[ct * 9 + j, nh].astype(np.float32)  # (128c,128n)
                out[128 * nh:128 * nh + 128] += Wtile.T @ xdq.astype(np.float32)
    out += bias.astype(np.float32)[:, None]
    return out.reshape(N_OUT, H, W)


# --------------------------------------------------------------------------
# bass kernel
# --------------------------------------------------------------------------
_CACHE = {}


def _build_nc():
    import concourse.bass as bass
    import concourse.bacc as bacc
    import concourse.mybir as mybir
    from concourse import tile

    f32, f16 = mybir.dt.float32, mybir.dt.float16
    ALU = mybir.AluOpType
    ACTF = mybir.ActivationFunctionType

    nc = bacc.Bacc("TRN2", target_bir_lowering=False, debug=False)
    xpad_d = nc.dram_tensor("xpad", (NCT, 128, NPW), f16, kind="ExternalInput")
    w_d = nc.dram_tensor("wt", (128, NCT * 9 * NH * 128), f16, kind="ExternalInput")
    e_d = nc.dram_tensor("et", (36, 2 * NCT * 128), f16, kind="ExternalInput")
    b_d = nc.dram_tensor("bias", (128, NH), f32, kind="ExternalInput")
    y_d = nc.dram_tensor("y", (NH, 128, L), f32, kind="ExternalOutput")

    es = ExitStack()
    with tile.TileContext(nc) as tc:
        pc = es.enter_context(tc.tile_pool(name="consts", bufs=1))
        pyps = es.enter_context(tc.tile_pool(name="yps", bufs=1, space="PSUM"))
        pexp = tc.alloc_tile_pool(name="expps", bufs=4, space="PSUM")
        pt = tc.alloc_tile_pool(name="Tslabs", bufs=1, side="right")
        pa = tc.alloc_tile_pool(name="arrays", bufs=1, side="right")
        pmm = es.enter_context(tc.tile_pool(name="mtmp", bufs=1))

        # ---- load constants -------------------------------------------------
        xp_sb = pc.tile([128, NCT, NPW], f16, tag="xp", name="xp")
        nc.sync.dma_start(out=xp_sb[:, 0, :], in_=xpad_d.ap()[0])
        nc.gpsimd.dma_start(out=xp_sb[:, 1, :], in_=xpad_d.ap()[1])
        w_sb = pc.tile([128, NCT * 9 * NH * 128], f16, tag="wsb", name="wsb")
        nc.sync.dma_start(out=w_sb[:], in_=w_d.ap())
        e_sb = pc.tile([36, 2 * NCT * 128], f16, tag="esb", name="esb")
        nc.sync.dma_start(out=e_sb[:], in_=e_d.ap())
        bias_sb = pc.tile([128, NH], f32, tag="bsb", name="bsb")
        nc.sync.dma_start(out=bias_sb[:], in_=b_d.ap())

        # ---- stage 1: image-domain max algebra (f16, ct-merged strided) -----
        arrs = {nm: pa.tile([128, NCT, NPW], f16, tag=nm, name=nm)
                for nm in ARR_NAMES}
        A, H2a, H3a, V2a, V3a = (arrs[n] for n in ARR_NAMES)
        # memset tails so slab gathers never read uninitialized cols
        nc.vector.memset(H2a[:, :, NPW - 1:], 0.0)
        nc.vector.memset(H3a[:, :, NPW - 2:], 0.0)
        nc.vector.memset(V2a[:, :, NPW - HP:], 0.0)
        nc.vector.memset(V3a[:, :, NPW - 2 * HP:], 0.0)
        nc.scalar.activation(A[:], xp_sb[:], ACTF.Abs)
        nc.vector.tensor_tensor(H2a[:, :, 0:NPW - 1], A[:, :, 0:NPW - 1],
                                A[:, :, 1:NPW], op=ALU.max)
        nc.vector.tensor_tensor(H3a[:, :, 0:NPW - 2], H2a[:, :, 0:NPW - 2],
                                A[:, :, 2:NPW], op=ALU.max)
        nc.vector.tensor_tensor(V2a[:, :, 0:NPW - HP], H3a[:, :, 0:NPW - HP],
                                H3a[:, :, HP:NPW], op=ALU.max)
        nc.vector.tensor_tensor(V3a[:, :, 0:NPW - 2 * HP], V2a[:, :, 0:NPW - 2 * HP],
                                H3a[:, :, 2 * HP:NPW], op=ALU.max)

        # ---- stage 2: slab gathers into granule space -----------------------
        T_sb = [pt.tile([NG, MCOLS], f16, tag=f"T{i}", name=f"T{i}")
                for i in range(6)]
        for i in range(6):
            nc.vector.memset(T_sb[i][:, SLAB:MCOLS], 0.0)
        # issue in array-readiness order (A first), alternate SP/Act issuers
        order = []
        for i in range(6):
            for gp in range(9):
                order.append((ARR_NAMES.index(TERMS[gp][i][1]), i, gp))
        order.sort()
        engs5 = [nc.sync, nc.gpsimd, nc.scalar]
        for n, (_, i, gp) in enumerate(order):
            cpp, arr, da, db = TERMS[gp][i]
            off = da * HP + db
            for ct in range(NCT):
                src = arrs[arr][cpp:cpp + 97:32, ct, off:off + SLAB]
                dst = T_sb[i][36 * ct + gp:36 * ct + gp + 28:9, 0:SLAB]
                engs5[(2 * n + ct) % 3].dma_start(out=dst, in_=src)
        pa.release()

        # ---- max tree (both halves) -----------------------------------------
        m16 = pmm.tile([NG, MCOLS], f16, tag="m16", name="m16")
        for h in range(2):
            hc = slice(h * HALF_MC, (h + 1) * HALF_MC)
            nc.vector.tensor_tensor(m16[:, hc], T_sb[0][:, hc], T_sb[1][:, hc],
                                    op=ALU.max)
            for i in range(2, 6):
                nc.vector.tensor_tensor(m16[:, hc], m16[:, hc], T_sb[i][:, hc],
                                        op=ALU.max)
        pt.release()

        # ---- stage 3 + expansion, per half ----------------------------------
        pm = es.enter_context(tc.tile_pool(name="scales", bufs=1))
        ps3 = tc.alloc_tile_pool(name="st3tmp", bufs=2, side="right")
        rj36 = pm.tile([36, NCT, MCOLS], f16, tag="rj36", name="rj36")
        sj36 = pm.tile([36, NCT, MCOLS], f16, tag="sj36", name="sj36")
        RJ = [pm.tile([128, NCT, MCOLS], f16, tag=f"RJ{p}", name=f"RJ{p}")
              for p in range(2)]
        SJ = [pm.tile([128, NCT, MCOLS], f16, tag=f"SJ{p}", name=f"SJ{p}")
              for p in range(2)]

        for h in range(2):
            hc = slice(h * HALF_MC, (h + 1) * HALF_MC)
            m32 = ps3.tile([NG, HALF_MC], f32, tag="m32", name="m32")
            rcp = ps3.tile([NG, HALF_MC], f32, tag="rcp", name="rcp")
            rj72 = ps3.tile([NG, HALF_MC], f16, tag="rj72", name="rj72")
            sj72 = ps3.tile([NG, HALF_MC], f16, tag="sj72", name="sj72")
            nc.vector.tensor_scalar(m32[:], m16[:, hc], float(M_CLAMP), None,
                                    op0=ALU.max)
            nc.vector.reciprocal_approx_fast(out=rcp[:], in_=m32[:])
            nc.vector.tensor_scalar(rj72[:], rcp[:], float(MAXQ),
                                    None, op0=ALU.mult)
            nc.vector.tensor_scalar(sj72[:], m16[:, hc], float(1.0 / MAXQ),
                                    float(M_CLAMP / MAXQ), op0=ALU.mult,
                                    op1=ALU.max)
            for (t72, t36) in ((rj72, rj36), (sj72, sj36)):
                for ct in range(NCT):
                    nc.gpsimd.dma_start(out=t36[:, ct, hc],
                                        in_=t72[36 * ct:36 * ct + 36, :])
            # expansion: j=0 state -> parity 0 arrays, j=1 -> parity 1
            for jst in range(2):
                for ct in range(NCT):
                    e_ap = e_sb[:, (jst * NCT + ct) * 128:(jst * NCT + ct + 1) * 128]
                    for (t36, ARRS) in ((rj36, RJ), (sj36, SJ)):
                        for p in range(4):
                            cl = slice(h * HALF_MC + p * EXP_P,
                                       h * HALF_MC + (p + 1) * EXP_P)
                            eps = pexp.tile([128, EXP_P], f32, tag="eps", name="eps")
                            nc.tensor.matmul(eps[:], e_ap, t36[:, ct, cl],
                                             start=True, stop=True)
                            nc.scalar.activation(ARRS[jst][:, ct, cl], eps[:],
                                                 ACTF.Copy)
        ps3.release()  # pmm (m16, 6.5 KB) is left allocated; feeds nothing after exp
        pexp.release()
        # PSUM banks freed by expansion let the last half run nh=1 inline
        pyps2 = es.enter_context(tc.tile_pool(name="yps2", bufs=1, space="PSUM"))

        # ---- main loop ------------------------------------------------------
        pxq = es.enter_context(tc.tile_pool(name="xdqstore", bufs=1))
        pw = es.enter_context(tc.tile_pool(name="work", bufs=2))
        pyo = es.enter_context(tc.tile_pool(name="yout", bufs=1))
        xp4 = xp_sb.rearrange("p ct (r c) -> p ct r c", r=HP)
        RJ4 = [a.rearrange("p ct (r c) -> p ct r c", r=H) for a in RJ]
        SJ4 = [a.rearrange("p ct (r c) -> p ct r c", r=H) for a in SJ]

        yps = [pyps.tile([128, SUB], f32, tag=f"y{s}", name=f"y{s}")
               for s in range(NSUB)]
        yps2 = [pyps2.tile([128, SUB], f32, tag=f"z{s}", name=f"z{s}")
                for s in range(NSUB)]
        xdq_st = [pxq.tile([128, NCT * HALF_L], f16, tag=f"xdq{i}", name=f"xdq{i}")
                  for i in range(9)]

        for h in range(2):
            r0 = HROWS * h
            hc = slice(h * HALF_MC, (h + 1) * HALF_MC)
            for j in range(9):
                par = j & 1
                dh, dw = divmod(j, 3)
                xv = xp4[:, :, r0 + dh:r0 + dh + HROWS, dw:dw + W]
                rjv = RJ4[par][:, :, r0:r0 + HROWS, 0:W]
                sjv = SJ4[par][:, :, r0:r0 + HROWS, 0:W]
                t = pw.tile([128, NCT * HALF_L], f16, tag="t", name="t")
                q = pw.tile([128, NCT * HALF_L], f16, tag="q", name="q")
                xdq = xdq_st[j]
                t4 = t.rearrange("p (ct a b) -> p ct a b", ct=NCT, a=HROWS)
                x4 = xdq.rearrange("p (ct a b) -> p ct a b", ct=NCT, a=HROWS)
                nc.vector.tensor_tensor(t4[:], xv, rjv, op=ALU.mult)
                nc.vector.tensor_scalar(q[:], t[:], float(F16C), -float(F16C),
                                        op0=ALU.add, op1=ALU.add)
                q4 = q.rearrange("p (ct a b) -> p ct a b", ct=NCT, a=HROWS)
                nc.vector.tensor_tensor(x4[:], q4[:], sjv, op=ALU.mult)
                for ct in range(NCT):
                    idx = ct * 9 + j
                    wsl = w_sb[:, (idx * NH) * 128:(idx * NH + 1) * 128]
                    for s in range(NSUB):
                        nc.tensor.matmul(
                            yps[s][:], wsl,
                            xdq[:, ct * HALF_L + s * SUB:ct * HALF_L + (s + 1) * SUB],
                            start=(j == 0 and ct == 0),
                            stop=(j == 8 and ct == 1))
                    if h == 1:
                        wsl1 = w_sb[:, (idx * NH + 1) * 128:(idx * NH + 2) * 128]
                        for s in range(NSUB):
                            nc.tensor.matmul(
                                yps2[s][:], wsl1,
                                xdq[:, ct * HALF_L + s * SUB:ct * HALF_L + (s + 1) * SUB],
                                start=(j == 0 and ct == 0),
                                stop=(j == 8 and ct == 1))
                # scale updates: advance parity-(j&1) arrays from state j to j+2
                if j <= 6:
                    for jp in (j + 1, j + 2):
                        cstar, rstar = FLIPS[jp]
                        for (t36, ARRS) in ((rj36, RJ), (sj36, SJ)):
                            nc.sync.dma_start(
                                out=ARRS[par][cstar:cstar + 97:32, :, hc],
                                in_=t36[rstar:rstar + 28:9, :, hc])
            # nh=0 evac + nh=1 (sweep for h=0, inline PSUM for h=1)
            ysb = [pyo.tile([128, HALF_L], f32, tag=f"ysb{nh}", name=f"ysb{nh}")
                   for nh in range(NH)]
            for s in range(NSUB):
                nc.scalar.activation(ysb[0][:, s * SUB:(s + 1) * SUB], yps[s][:],
                                     ACTF.Identity, bias=bias_sb[:, 0:1], scale=1.0)
            if h == 0:
                for ct in range(NCT):
                    for j in range(9):
                        idx = ct * 9 + j
                        wsl = w_sb[:, (idx * NH + 1) * 128:(idx * NH + 2) * 128]
                        for s in range(NSUB):
                            nc.tensor.matmul(
                                yps[s][:], wsl,
                                xdq_st[j][:, ct * HALF_L + s * SUB:
                                          ct * HALF_L + (s + 1) * SUB],
                                start=(idx == 0), stop=(idx == 17))
                ynh1 = yps
            else:
                ynh1 = yps2
            for s in range(NSUB):
                nc.scalar.activation(ysb[1][:, s * SUB:(s + 1) * SUB], ynh1[s][:],
                                     ACTF.Identity, bias=bias_sb[:, 1:2], scale=1.0)
            for nh in range(NH):
                for yhh in range(2):
                    c0, c1 = yhh * 2 * SUB, (yhh + 1) * 2 * SUB
                    nc.scalar.dma_start(
                        out=y_d.ap()[nh, :, h * HALF_L + c0:h * HALF_L + c1],
                        in_=ysb[nh][:, c0:c1])
        es.close()
    nc.compile()
    return nc


def kernel(input, weight, bias):
    input = np.asarray(input, np.float32)
    weight = np.asarray(weight, np.float32)
    bias = np.asarray(bias, np.float32)
    B = input.shape[0]
    assert B == 8 and input.shape[1:] == (C_IN, H, W)

    from concourse import bass_utils

    if "nc" not in _CACHE:
        _CACHE["nc"] = _build_nc()
    nc = _CACHE["nc"]

    in_maps = make_in_maps(input, weight, bias)
    res = bass_utils.run_bass_kernel_spmd(nc, in_maps, core_ids=list(range(B)))
    out = np.stack([r["y"].reshape(N_OUT, H, W) for r in res.results])
    return out.astype(np.float32)


if __name__ == "__main__":
    pass


# revision 27
# speedup vs baseline: 1.0329x; 1.0329x over previous
"""Trainium2 Bass kernel for nn_Conv2dMem (bit-slice fake-quantized 3x3 conv).

Math (per image): unfold 3x3/pad1 -> per-row granule (32 along K, C-major)
symmetric int7 fake-quant of activations; per 32x32 block fake-quant of
weights; GEMM; bias.

Strategy (8 cores, batch-parallel, 1 image/core), v2 "all-f16 pipeline":
  - Weights fake-quantized exactly on host (numpy), fed as fp16 GEMM tiles.
  - Image stays in (C x 58*58) padded layout; conv = 18 shifted GEMM
    accumulations (2 channel-tiles x 9 kernel positions) into PSUM.
  - Granule absmax m[g, l] built on-device from image-domain max algebra
    (A/H2/H3/V2/V3 window maxes, all f16 DVE at 2x rate) + contiguous-slab
    partition-strided gather DMAs into granule space (72, 56*58) + max tree.
  - Scales rj = 63*recip(m), sj = m/63 (f16), expanded granule->channel ONCE
    per j-parity (j=0 and j=1 states) via 0/1 E-matmuls + scalar-engine evac.
    As j advances, only 4 channels per 128 change granule row; those rows are
    patched with small strided DMAs (granule row stride 9 -> channel stride 32).
  - Quantize per (j, ct, half-L): t = x*rj (f16 tensor_tensor, 2x),
    q = (t+1536)-1536 (f16 tensor_scalar, 4x; f16 magic RNE round),
    xdq = q*sj (f16 tensor_tensor, 2x). xdq stored per idx; GEMM runs nh=0
    inline and nh=1 as a sweep at half end so PSUM fits in 4 banks and the
    next half's scale expansion can overlap.
"""
import numpy as np
from contextlib import ExitStack

C_IN = 256
N_OUT = 256
H = W = 56
HP = WP = 58
NPW = HP * WP            # 3364
L = H * W                # 3136
KS = 3
GRAN = 32
NG = (C_IN * KS * KS) // GRAN   # 72 granules
MAXQ = 63.0
F16C = 1536.0            # f16 magic rounding constant (RNE to integer)
NCT = 2                  # channel partition tiles (256/128)
NH = 2                   # output-channel halves
M_CLAMP = 6e-5           # fp16-safe clamp for zero-granule guard

MCOLS = H * HP           # 3248: m/scale arrays, 58-pitch rows x 56 rows
SLAB = MCOLS - 2         # 3246: slab gather length (stays in-bounds)
HROWS = 28               # image rows per half
HALF_MC = HROWS * HP     # 1624 scale cols per half
HALF_L = HROWS * W       # 1568 packed output cols per half
SUB = 392                # PSUM sub-chunk (7 rows of 56)
NSUB = 4
EXP_P = 406              # expansion piece cols (1624/4)

ARR_NAMES = ("A", "H2", "H3", "V2", "V3")


# --------------------------------------------------------------------------
# host-side index tables
# --------------------------------------------------------------------------
def granule_terms():
    """For each g' in [0,9): list of (c'', arr, da, db) whose pointwise max
    over terms equals the granule absmax. Padded to 6 terms (repeats)."""
    out = []
    for gp in range(9):
        c_lo, c_hi = (32 * gp) // 9, (32 * gp + 31) // 9
        j0 = 32 * gp - 9 * c_lo
        j1 = 32 * gp + 32 - 9 * c_hi
        terms = []
        if j0 == 0:
            terms.append((c_lo, "V3", 0, 0))
        else:
            dh0, dw0 = divmod(j0, 3)
            terms.append((c_lo, {0: "H3", 1: "H2", 2: "A"}[dw0], dh0, dw0))
            if dh0 == 0:
                terms.append((c_lo, "V2", 1, 0))
            elif dh0 == 1:
                terms.append((c_lo, "H3", 2, 0))
        for c in range(c_lo + 1, c_hi):
            terms.append((c, "V3", 0, 0))
        if j1 == 9:
            terms.append((c_hi, "V3", 0, 0))
        else:
            q, rr = divmod(j1, 3)
            if q == 1:
                terms.append((c_hi, "H3", 0, 0))
            elif q == 2:
                terms.append((c_hi, "V2", 0, 0))
            if rr == 1:
                terms.append((c_hi, "A", q, 0))
            elif rr == 2:
                terms.append((c_hi, "H2", q, 0))
        assert 1 <= len(terms) <= 6, (gp, terms)
        while len(terms) < 6:
            terms.append(terms[0])
        out.append(terms)
    return out


TERMS = granule_terms()


def flip_table():
    """For jp in 1..8: (c*, r*): channels c* + 32t flip to granule row
    r* + 9t (+36ct via the ct free dim) when j reaches jp."""
    out = {}
    for jp in range(1, 9):
        cstar = (-25 * jp) % 32          # 9^-1 = 25 (mod 32)
        rstar = (9 * cstar + jp) // 32
        assert (9 * cstar + jp) % 32 == 0 and 0 <= rstar < 9
        out[jp] = (cstar, rstar)
    return out


FLIPS = flip_table()


def build_E2():
    """E[jst, ct] : (36, 128) f16; expands granule rows to channel rows for
    the j=jst state."""
    E = np.zeros((2, NCT, 36, 128), np.float16)
    for jst in range(2):
        for ct in range(NCT):
            for p in range(128):
                g = (9 * (128 * ct + p) + jst) // 32
                E[jst, ct, g - 36 * ct, p] = 1.0
    return E


def quantize_weight_host(weight):
    """Exact numpy replica of reference _fake_quant_weight on w2d=(K,N)."""
    w2d = weight.reshape(N_OUT, -1).T.astype(np.float32)      # (2304, 256)
    K, N = w2d.shape
    wg = w2d.reshape(K // 32, 32, N // 32, 32)
    max_abs = np.max(np.abs(wg), axis=(1, 3), keepdims=True)
    scale = (max_abs / np.float32(MAXQ)).astype(np.float32)
    scale = np.where(scale == 0, np.float32(1.0), scale)
    q = np.clip(np.round(wg / scale), -MAXQ, MAXQ)
    deq = (q * scale).astype(np.float32).reshape(K, N)
    return deq


def pack_weights(wdq):
    """(2304, 256) -> W[idx=ct*9+j, nh, p, n] fp16 stationary tiles."""
    Wt = np.zeros((NCT * 9, NH, 128, 128), np.float16)
    for ct in range(NCT):
        for j in range(9):
            rows = (9 * (128 * ct + np.arange(128)) + j)      # (128,)
            for nh in range(NH):
                Wt[ct * 9 + j, nh] = wdq[rows][:, 128 * nh:128 * nh + 128]
    return Wt


def pad_image(x):
    """(256,56,56) fp32 -> (2,128,3364) fp16 padded."""
    xp = np.pad(x, ((0, 0), (1, 1), (1, 1))).astype(np.float16)
    return xp.reshape(NCT, 128, NPW)


def make_in_maps(input, weight, bias):
    wdq = quantize_weight_host(np.asarray(weight, np.float32))
    Wt = np.ascontiguousarray(np.transpose(pack_weights(wdq), (2, 0, 1, 3))
                              ).reshape(128, NCT * 9 * NH * 128)
    E = np.ascontiguousarray(np.transpose(build_E2(), (2, 0, 1, 3))
                             ).reshape(36, 2 * NCT * 128)
    b = np.ascontiguousarray(np.asarray(bias, np.float32).reshape(NH, 128).T
                             ).astype(np.float32)
    x32 = np.asarray(input, np.float32)
    return [{"xpad": pad_image(x32[bi]), "wt": Wt, "et": E, "bias": b}
            for bi in range(x32.shape[0])]


# --------------------------------------------------------------------------
# numpy model of the device pipeline (for validation in test.py)
# --------------------------------------------------------------------------
def model_core(x, Wt, bias):
    """Bit-accurate-ish numpy model of the bass kernel for one image.
    x: (256,56,56) fp32. Returns (256,56,56) fp32."""
    f16 = np.float16
    xp16 = pad_image(x).reshape(C_IN, HP, WP)                # f16
    A = np.abs(xp16)
    H2 = np.maximum(A[:, :, :-1], A[:, :, 1:])               # (C,58,57)
    H3 = np.maximum(H2[:, :, :-1], A[:, :, 2:])              # (C,58,56)
    V2 = np.maximum(H3[:, :-1], H3[:, 1:])                   # (C,57,56)
    V3 = np.maximum(V2[:, :-1], H3[:, 2:])                   # (C,56,56)
    arrs = {"A": A, "H2": H2, "H3": H3, "V2": V2, "V3": V3}

    m = np.zeros((NG, H, W), f16)
    for g in range(NG):
        B, gp = divmod(g, 9)
        acc = None
        for (cpp, arr, da, db) in TERMS[gp]:
            c = 32 * B + cpp
            v = arrs[arr][c, da:da + H, db:db + W]
            acc = v if acc is None else np.maximum(acc, v)
        m[g] = acc

    m32 = np.maximum(m.astype(np.float32), np.float32(M_CLAMP))
    rj = (np.float32(MAXQ) * (1.0 / m32)).astype(f16)        # (72,56,56)
    sj = np.maximum(m.astype(np.float32) * np.float32(1.0 / MAXQ),
                    np.float32(M_CLAMP / MAXQ)).astype(f16)

    out = np.zeros((N_OUT, L), np.float32)
    for ct in range(NCT):
        for j in range(9):
            dh, dw = divmod(j, 3)
            cs = np.arange(128 * ct, 128 * ct + 128)
            g = (9 * cs + j) // 32                            # (128,)
            xv = xp16[cs, dh:dh + H, dw:dw + W].reshape(128, L)
            # f16 product, f16 magic round, f16 dequant -- matches device
            t = (xv * rj[g].reshape(128, L)).astype(f16)
            u = (t.astype(f16) + f16(F16C)).astype(f16)
            q = (u.astype(np.float32) - np.float32(F16C)).astype(f16)
            xdq = (q.astype(np.float32) * sj[g].reshape(128, L).astype(np.float32)).astype(f16)
            for nh in range(NH):
                Wtile = Wt[ct * 9 + j, nh].astype(np.float32)  # (128c,128n)
                out[128 * nh:128 * nh + 128] += Wtile.T @ xdq.astype(np.float32)
    out += bias.astype(np.float32)[:, None]
    return out.reshape(N_OUT, H, W)


# --------------------------------------------------------------------------
# bass kernel
# --------------------------------------------------------------------------
_CACHE = {}


def _build_nc():
    import concourse.bass as bass
    import concourse.bacc as bacc
    import concourse.mybir as mybir
    from concourse import tile

    f32, f16 = mybir.dt.float32, mybir.dt.float16
    ALU = mybir.AluOpType
    ACTF = mybir.ActivationFunctionType

    nc = bacc.Bacc("TRN2", target_bir_lowering=False, debug=False)
    xpad_d = nc.dram_tensor("xpad", (NCT, 128, NPW), f16, kind="ExternalInput")
    w_d = nc.dram_tensor("wt", (128, NCT * 9 * NH * 128), f16, kind="ExternalInput")
    e_d = nc.dram_tensor("et", (36, 2 * NCT * 128), f16, kind="ExternalInput")
    b_d = nc.dram_tensor("bias", (128, NH), f32, kind="ExternalInput")
    y_d = nc.dram_tensor("y", (NH, 128, L), f32, kind="ExternalOutput")

    es = ExitStack()
    with tile.TileContext(nc) as tc:
        pc = es.enter_context(tc.tile_pool(name="consts", bufs=1))
        pyps = es.enter_context(tc.tile_pool(name="yps", bufs=1, space="PSUM"))
        pexp = tc.alloc_tile_pool(name="expps", bufs=4, space="PSUM")
        pt = tc.alloc_tile_pool(name="Tslabs", bufs=1, side="right")
        pa = tc.alloc_tile_pool(name="arrays", bufs=1, side="right")
        pmm = es.enter_context(tc.tile_pool(name="mtmp", bufs=1))

        # ---- load constants -------------------------------------------------
        xp_sb = pc.tile([128, NCT, NPW], f16, tag="xp", name="xp")
        for ct in range(NCT):
            nc.sync.dma_start(out=xp_sb[:, ct, :], in_=xpad_d.ap()[ct])
        w_sb = pc.tile([128, NCT * 9 * NH * 128], f16, tag="wsb", name="wsb")
        nc.sync.dma_start(out=w_sb[:], in_=w_d.ap())
        e_sb = pc.tile([36, 2 * NCT * 128], f16, tag="esb", name="esb")
        nc.sync.dma_start(out=e_sb[:], in_=e_d.ap())
        bias_sb = pc.tile([128, NH], f32, tag="bsb", name="bsb")
        nc.sync.dma_start(out=bias_sb[:], in_=b_d.ap())

        # ---- stage 1: image-domain max algebra (f16, ct-merged strided) -----
        arrs = {nm: pa.tile([128, NCT, NPW], f16, tag=nm, name=nm)
                for nm in ARR_NAMES}
        A, H2a, H3a, V2a, V3a = (arrs[n] for n in ARR_NAMES)
        # memset tails so slab gathers never read uninitialized cols
        nc.vector.memset(H2a[:, :, NPW - 1:], 0.0)
        nc.vector.memset(H3a[:, :, NPW - 2:], 0.0)
        nc.vector.memset(V2a[:, :, NPW - HP:], 0.0)
        nc.vector.memset(V3a[:, :, NPW - 2 * HP:], 0.0)
        nc.scalar.activation(A[:], xp_sb[:], ACTF.Abs)
        nc.vector.tensor_tensor(H2a[:, :, 0:NPW - 1], A[:, :, 0:NPW - 1],
                                A[:, :, 1:NPW], op=ALU.max)
        nc.vector.tensor_tensor(H3a[:, :, 0:NPW - 2], H2a[:, :, 0:NPW - 2],
                                A[:, :, 2:NPW], op=ALU.max)
        nc.vector.tensor_tensor(V2a[:, :, 0:NPW - HP], H3a[:, :, 0:NPW - HP],
                                H3a[:, :, HP:NPW], op=ALU.max)
        nc.vector.tensor_tensor(V3a[:, :, 0:NPW - 2 * HP], V2a[:, :, 0:NPW - 2 * HP],
                                H3a[:, :, 2 * HP:NPW], op=ALU.max)

        # ---- stage 2: slab gathers into granule space -----------------------
        T_sb = [pt.tile([NG, MCOLS], f16, tag=f"T{i}", name=f"T{i}")
                for i in range(6)]
        for i in range(6):
            nc.vector.memset(T_sb[i][:, SLAB:MCOLS], 0.0)
        # issue in array-readiness order (A first), alternate SP/Act issuers
        order = []
        for i in range(6):
            for gp in range(9):
                order.append((ARR_NAMES.index(TERMS[gp][i][1]), i, gp))
        order.sort()
        engs5 = [nc.sync, nc.gpsimd, nc.scalar]
        for n, (_, i, gp) in enumerate(order):
            cpp, arr, da, db = TERMS[gp][i]
            off = da * HP + db
            for ct in range(NCT):
                src = arrs[arr][cpp:cpp + 97:32, ct, off:off + SLAB]
                dst = T_sb[i][36 * ct + gp:36 * ct + gp + 28:9, 0:SLAB]
                engs5[(2 * n + ct) % 3].dma_start(out=dst, in_=src)
        pa.release()

        # ---- max tree (both halves) -----------------------------------------
        m16 = pmm.tile([NG, MCOLS], f16, tag="m16", name="m16")
        for h in range(2):
            hc = slice(h * HALF_MC, (h + 1) * HALF_MC)
            nc.vector.tensor_tensor(m16[:, hc], T_sb[0][:, hc], T_sb[1][:, hc],
                                    op=ALU.max)
            for i in range(2, 6):
                nc.vector.tensor_tensor(m16[:, hc], m16[:, hc], T_sb[i][:, hc],
                                        op=ALU.max)
        pt.release()

        # ---- stage 3 + expansion, per half ----------------------------------
        pm = es.enter_context(tc.tile_pool(name="scales", bufs=1))
        ps3 = tc.alloc_tile_pool(name="st3tmp", bufs=2, side="right")
        rj36 = pm.tile([36, NCT, MCOLS], f16, tag="rj36", name="rj36")
        sj36 = pm.tile([36, NCT, MCOLS], f16, tag="sj36", name="sj36")
        RJ = [pm.tile([128, NCT, MCOLS], f16, tag=f"RJ{p}", name=f"RJ{p}")
              for p in range(2)]
        SJ = [pm.tile([128, NCT, MCOLS], f16, tag=f"SJ{p}", name=f"SJ{p}")
              for p in range(2)]

        for h in range(2):
            hc = slice(h * HALF_MC, (h + 1) * HALF_MC)
            m32 = ps3.tile([NG, HALF_MC], f32, tag="m32", name="m32")
            rcp = ps3.tile([NG, HALF_MC], f32, tag="rcp", name="rcp")
            rj72 = ps3.tile([NG, HALF_MC], f16, tag="rj72", name="rj72")
            sj72 = ps3.tile([NG, HALF_MC], f16, tag="sj72", name="sj72")
            nc.vector.tensor_scalar(m32[:], m16[:, hc], float(M_CLAMP), None,
                                    op0=ALU.max)
            nc.vector.reciprocal_approx_fast(out=rcp[:], in_=m32[:])
            nc.vector.tensor_scalar(rj72[:], rcp[:], float(MAXQ),
                                    None, op0=ALU.mult)
            nc.vector.tensor_scalar(sj72[:], m16[:, hc], float(1.0 / MAXQ),
                                    float(M_CLAMP / MAXQ), op0=ALU.mult,
                                    op1=ALU.max)
            for (t72, t36) in ((rj72, rj36), (sj72, sj36)):
                for ct in range(NCT):
                    nc.gpsimd.dma_start(out=t36[:, ct, hc],
                                        in_=t72[36 * ct:36 * ct + 36, :])
            # expansion: j=0 state -> parity 0 arrays, j=1 -> parity 1
            for jst in range(2):
                for ct in range(NCT):
                    e_ap = e_sb[:, (jst * NCT + ct) * 128:(jst * NCT + ct + 1) * 128]
                    for (t36, ARRS) in ((rj36, RJ), (sj36, SJ)):
                        for p in range(4):
                            cl = slice(h * HALF_MC + p * EXP_P,
                                       h * HALF_MC + (p + 1) * EXP_P)
                            eps = pexp.tile([128, EXP_P], f32, tag="eps", name="eps")
                            nc.tensor.matmul(eps[:], e_ap, t36[:, ct, cl],
                                             start=True, stop=True)
                            nc.scalar.activation(ARRS[jst][:, ct, cl], eps[:],
                                                 ACTF.Copy)
        ps3.release()  # pmm (m16, 6.5 KB) is left allocated; feeds nothing after exp
        pexp.release()
        # PSUM banks freed by expansion let the last half run nh=1 inline
        pyps2 = es.enter_context(tc.tile_pool(name="yps2", bufs=1, space="PSUM"))

        # ---- main loop ------------------------------------------------------
        pxq = es.enter_context(tc.tile_pool(name="xdqstore", bufs=1))
        pw = es.enter_context(tc.tile_pool(name="work", bufs=2))
        pyo = es.enter_context(tc.tile_pool(name="yout", bufs=1))
        xp4 = xp_sb.rearrange("p ct (r c) -> p ct r c", r=HP)
        RJ4 = [a.rearrange("p ct (r c) -> p ct r c", r=H) for a in RJ]
        SJ4 = [a.rearrange("p ct (r c) -> p ct r c", r=H) for a in SJ]

        yps = [pyps.tile([128, SUB], f32, tag=f"y{s}", name=f"y{s}")
               for s in range(NSUB)]
        yps2 = [pyps2.tile([128, SUB], f32, tag=f"z{s}", name=f"z{s}")
                for s in range(NSUB)]
        xdq_st = [pxq.tile([128, NCT * HALF_L], f16, tag=f"xdq{i}", name=f"xdq{i}")
                  for i in range(9)]

        for h in range(2):
            r0 = HROWS * h
            hc = slice(h * HALF_MC, (h + 1) * HALF_MC)
            for j in range(9):
                par = j & 1
                dh, dw = divmod(j, 3)
                xv = xp4[:, :, r0 + dh:r0 + dh + HROWS, dw:dw + W]
                rjv = RJ4[par][:, :, r0:r0 + HROWS, 0:W]
                sjv = SJ4[par][:, :, r0:r0 + HROWS, 0:W]
                t = pw.tile([128, NCT * HALF_L], f16, tag="t", name="t")
                q = pw.tile([128, NCT * HALF_L], f16, tag="q", name="q")
                xdq = xdq_st[j]
                t4 = t.rearrange("p (ct a b) -> p ct a b", ct=NCT, a=HROWS)
                x4 = xdq.rearrange("p (ct a b) -> p ct a b", ct=NCT, a=HROWS)
                nc.vector.tensor_tensor(t4[:], xv, rjv, op=ALU.mult)
                if j <= 6:
                    for jp in (j + 1, j + 2):
                        cstar, rstar = FLIPS[jp]
                        nc.sync.dma_start(
                            out=RJ[par][cstar:cstar + 97:32, :, hc],
                            in_=rj36[rstar:rstar + 28:9, :, hc])
                nc.vector.tensor_scalar(q[:], t[:], float(F16C), -float(F16C),
                                        op0=ALU.add, op1=ALU.add)
                q4 = q.rearrange("p (ct a b) -> p ct a b", ct=NCT, a=HROWS)
                nc.vector.tensor_tensor(x4[:], q4[:], sjv, op=ALU.mult)
                if j <= 6:
                    for jp in (j + 1, j + 2):
                        cstar, rstar = FLIPS[jp]
                        nc.sync.dma_start(
                            out=SJ[par][cstar:cstar + 97:32, :, hc],
                            in_=sj36[rstar:rstar + 28:9, :, hc])
                for ct in range(NCT):
                    idx = ct * 9 + j
                    wsl = w_sb[:, (idx * NH) * 128:(idx * NH + 1) * 128]
                    for s in range(NSUB):
                        nc.tensor.matmul(
                            yps[s][:], wsl,
                            xdq[:, ct * HALF_L + s * SUB:ct * HALF_L + (s + 1) * SUB],
                            start=(j == 0 and ct == 0),
                            stop=(j == 8 and ct == 1))
                    if h == 1:
                        wsl1 = w_sb[:, (idx * NH + 1) * 128:(idx * NH + 2) * 128]
                        for s in range(NSUB):
                            nc.tensor.matmul(
                                yps2[s][:], wsl1,
                                xdq[:, ct * HALF_L + s * SUB:ct * HALF_L + (s + 1) * SUB],
                                start=(j == 0 and ct == 0),
                                stop=(j == 8 and ct == 1))
            # nh=0 evac + nh=1 (sweep for h=0, inline PSUM for h=1)
            ysb = [pyo.tile([128, HALF_L], f32, tag=f"ysb{nh}", name=f"ysb{nh}")
                   for nh in range(NH)]
            for s in range(NSUB):
                nc.scalar.activation(ysb[0][:, s * SUB:(s + 1) * SUB], yps[s][:],
                                     ACTF.Identity, bias=bias_sb[:, 0:1], scale=1.0)
            if h == 0:
                for ct in range(NCT):
                    for j in range(9):
                        idx = ct * 9 + j
                        wsl = w_sb[:, (idx * NH + 1) * 128:(idx * NH + 2) * 128]
                        for s in range(NSUB):
                            nc.tensor.matmul(
                                yps[s][:], wsl,
                                xdq_st[j][:, ct * HALF_L + s * SUB:
                                          ct * HALF_L + (s + 1) * SUB],
                                start=(idx == 0), stop=(idx == 17))
                ynh1 = yps
            else:
                ynh1 = yps2
            for s in range(NSUB):
                nc.scalar.activation(ysb[1][:, s * SUB:(s + 1) * SUB], ynh1[s][:],
                                     ACTF.Identity, bias=bias_sb[:, 1:2], scale=1.0)
            for nh in range(NH):
                for yhh in range(2):
                    c0, c1 = yhh * 2 * SUB, (yhh + 1) * 2 * SUB
                    nc.scalar.dma_start(
                        out=y_d.ap()[nh, :, h * HALF_L + c0:h * HALF_L + c1],
                        in_=ysb[nh][:, c0:c1])
        es.close()
    nc.compile()
    return nc


def kernel(input, weight, bias):
    input = np.asarray(input, np.float32)
    weight = np.asarray(weight, np.float32)
    bias = np.asarray(bias, np.float32)
    B = input.shape[0]
    assert B == 8 and input.shape[1:] == (C_IN, H, W)

    from concourse import bass_utils

    if "nc" not in _CACHE:
        _CACHE["nc"] = _build_nc()
    nc = _CACHE["nc"]

    in_maps = make_in_maps(input, weight, bias)
    res = bass_utils.run_bass_kernel_spmd(nc, in_maps, core_ids=list(range(B)))
    out = np.stack([r["y"].reshape(N_OUT, H, W) for r in res.results])
    return out.astype(np.float32)


if __name__ == "__main__":
    pass
